# revision 15
# baseline (speedup 1.0000x reference)
"""Trainium2 Bass kernel for a dense transformer encoder layer.

Problem: B=1, S=4096, D=512, F=2048, H=8 heads (Dh=64), fp32 reference,
attention WITHOUT 1/sqrt(Dh) scaling, int mask (0 -> -1e9 before softmax),
two LayerNorms, ReLU FFN.

Sharding (query/row-parallel, no collectives): every core redundantly
computes the full kT = (x@wk).T and v = x@wv, plus its own 512-query shard.

Pipelined schedule (v2): the kT/v projections are software-pipelined INTO
attention pass A (heads 0-3): per 128-key chunk the PE queue interleaves
[score pair] [4 projection matmuls] so the ACT exp latency of the previous
score pair hides behind projection work and the PE never idles.  PSUM: 4
banks hold the 4 attnV accumulators (V gets a ones column so softmax
denominators are free), the other 4 banks are a 2-deep rotating pool shared
by score pairs and projection chunks.  Pass B (heads 4-7) is ACT-bound by
construction (exp is 1 elem/cycle/lane); it uses a 4-bank score tile so exp
runs as one [128,2048] instruction, and the FFN weight DMAs + residual
pre-adds ride in its shadow.  Softmax reciprocals and LayerNorm rstd use
the fast custom-DVE reciprocal (no giant single-lane RECIPROCAL ops, no
extra ACT table loads).  LayerNorm statistics matmuls are interleaved with
the producing projection loops.

dtypes: fp16 (tf32-grade) for QKV/FFN weights+activations and scores; bf16
for exp outputs / V / mask (exp needs the bf16 exponent range); fp32r for
K=1 broadcast matmuls and LN/residual arithmetic; fp32 accumulation in PSUM.
"""

import numpy as np
import ml_dtypes

import concourse.bass as bass
import concourse.bacc as bacc
import concourse.tile as tile
from concourse import mybir
from concourse.bass import ts, ds
from concourse.bass_utils import run_bass_kernel_spmd

AF = mybir.ActivationFunctionType
F32 = mybir.dt.float32
DT = mybir.dt.float32r  # fp32 storage, single-pass PE mode
DT16 = mybir.dt.float16
BF16 = mybir.dt.bfloat16

N_CORES = 8
EPS = 1e-5


def build_encoder_kernel(nc, S=4096, D=512, F=2048, H=8, n_cores=8):
    P = 128
    SH = S // n_cores          # query shard per core
    DC = D // P                # feature chunks of 128
    FC = F // P                # ffn chunks of 128
    TB = S // 512              # 512-wide t blocks
    TC = S // P                # 128-tall t chunks
    Dh = D // H
    assert Dh == 64 and DC * P == D

    d = lambda name, shape, dt: nc.dram_tensor(name, shape, dt, kind="ExternalInput").ap()
    xT = d("xT", [D, S], DT16)
    xsT = d("xsT", [D, SH], DT)
    xs16 = d("xs16", [D, SH], DT16)
    maskT = d("maskT", [S, SH], BF16)
    wq, wk, wv, wo = (d(n, [D, D], DT16) for n in ("wq", "wk", "wv", "wo"))
    w1 = d("w1", [D, F], DT16)
    w2 = d("w2", [F, D], DT16)
    bq, bk, bo = (d(n, [D], F32) for n in ("bq", "bk", "bo"))
    bv = d("bv", [D], DT)
    b1 = d("b1", [F], F32)
    b2 = d("b2", [D], F32)
    g1, be1, g2, be2 = (d(n, [D], DT) for n in ("g1", "be1", "g2", "be2"))
    ones = d("ones", [512], DT)
    outT = nc.dram_tensor("outT", [D, SH], F32, kind="ExternalOutput").ap()

    with tile.TileContext(nc) as tc:
        _emit(nc, tc, locals())


def _emit(nc, tc, io):
    P = 128
    xT, xsT, maskT = io["xT"], io["xsT"], io["maskT"]
    outT = io["outT"]
    S, D, F, H = io["S"], io["D"], io["F"], io["H"]
    SH, DC, FC, TB, TC, Dh = io["SH"], io["DC"], io["FC"], io["TB"], io["TC"], io["Dh"]
    HPC = P // Dh              # heads per 128-feature chunk (2)

    from contextlib import ExitStack
    with ExitStack() as root:
        # ---- global pools (live for the whole kernel) ----
        gconst = root.enter_context(tc.tile_pool(name="gconst", bufs=1))
        gbig = root.enter_context(tc.tile_pool(name="gbig", bufs=1))

        ones_row = gconst.tile([1, P], DT)
        nc.sync.dma_start(out=ones_row, in_=io["ones"][None, :P])
        ones_col = gconst.tile([P, 1], DT)
        nc.sync.dma_start(out=ones_col, in_=io["ones"][:P, None])
        eps_sb = gconst.tile([1, 1], F32)
        nc.vector.memset(eps_sb, EPS)

        xs_sb = gbig.tile([P, DC, SH], DT)        # own x shard (residual)
        nc.sync.dma_start(out=xs_sb, in_=xsT.rearrange("(c p) s -> p c s", p=P))
        xs16_sb = gbig.tile([P, DC, SH], DT16)    # fp16 twin for the q projection
        nc.sync.dma_start(out=xs16_sb, in_=io["xs16"].rearrange("(c p) s -> p c s", p=P))
        attn_sb = gbig.tile([P, DC, SH], DT16)    # normalized attention out^T

        def load_vec(name, chunks):               # (n,) -> [128, chunks]
            t = gconst.tile([P, chunks], F32, tag=f"vec_{name}", name=f"vec_{name}")
            nc.sync.dma_start(out=t, in_=io[name].rearrange("(c p) -> p c", p=P))
            return t

        bq_sb, bk_sb, bo_sb = (load_vec(n, DC) for n in ("bq", "bk", "bo"))
        bv_row = gconst.tile([1, D], DT)
        nc.sync.dma_start(out=bv_row, in_=io["bv"][None, :])
        b1_sb = load_vec("b1", FC)
        b2_sb = load_vec("b2", DC)

        def load_row(name):                       # (n,) -> [1, n] f32r row
            t = gconst.tile([1, D], DT, tag=f"row_{name}", name=f"row_{name}")
            nc.sync.dma_start(out=t, in_=io[name][None, :])
            return t

        g1_row, be1_row, g2_row, be2_row = (load_row(n) for n in ("g1", "be1", "g2", "be2"))
        ones_s = gconst.tile([1, SH], DT)
        nc.sync.dma_start(out=ones_s, in_=io["ones"][None, :SH])

        # pass-B riders / early phase-3 state (lives to the end; opened below
        # the mid pools so pool open/close stays LIFO)
        pbr = root.enter_context(tc.tile_pool(name="pbr", bufs=1))

        # ---- mid-lived pools: attention state + projection weights ----
        mid = ExitStack()
        kqv = mid.enter_context(tc.tile_pool(name="kqv", bufs=1))
        pw = mid.enter_context(tc.tile_pool(name="pw", bufs=1))
        p1x = mid.enter_context(tc.tile_pool(name="p1x", bufs=2))
        p2m = mid.enter_context(tc.tile_pool(name="p2m", bufs=3))
        amp = mid.enter_context(tc.tile_pool(name="amp", bufs=3))
        p2s = mid.enter_context(tc.tile_pool(name="p2s", bufs=4))

        kT_sb = kqv.tile([P, DC, S], DT16)        # (x@wk)^T, full sequence
        qT_sb = kqv.tile([P, DC, SH], DT16)       # (xs@wq)^T
        v_sb = kqv.tile([P, TC, H, Dh + 1], BF16)  # v chunks + ones column
        nc.vector.memset(v_sb[:, :, :, Dh:Dh + 1], 1.0)

        # PE warmup on the first-arriving input
        with tc.tile_pool(name="warmps", bufs=1, space="PSUM") as warmps:
            wps = warmps.tile([1, SH], F32)
            for _ in range(8):
                nc.tensor.matmul(wps, lhsT=ones_col, rhs=xs_sb[:, 0, :],
                                 start=True, stop=True)

        def load_w(name):                         # (D, n) -> [128, DC, n]
            w = io[name]
            t = pw.tile([P, DC, w.shape[1]], DT16, tag=f"w_{name}", name=f"w_{name}")
            nc.sync.dma_start(out=t, in_=w.rearrange("(c p) n -> p c n", p=P))
            return t

        # first t-block of xT transfers early
        xt_cur = []
        for ci in range(DC):
            t = p1x.tile([P, 512], DT16, tag=f"xt{ci}", name=f"xt0_{ci}")
            nc.sync.dma_start(out=t, in_=xT[ds(ci * P, P), ts(0, 512)])
            xt_cur.append(t)

        wq_sb, wk_sb, wv_sb = load_w("wq"), load_w("wk"), load_w("wv")

        mask_tiles = {}

        def mask_dma(tig):
            if tig in mask_tiles or tig >= TC:
                return
            m = p2m.tile([P, SH], BF16, tag="mask")
            nc.sync.dma_start(out=m, in_=maskT[ds(tig * P, P), :])
            mask_tiles[tig] = m

        # ============ merged phase: qT/kT/v projections + attention pass A ====
        mix_ctx = tc.tile_pool(name="mixps", bufs=2, space="PSUM")
        mix = mix_ctx.__enter__()
        outA_ctx = tc.tile_pool(name="outA", bufs=1, space="PSUM")
        outA = outA_ctx.__enter__()
        headsA = [0, 1, 2, 3]
        out_ps = {h: outA.tile([Dh + 1, SH], F32, tag=f"out{h}", name=f"out_ps{h}")
                  for h in headsA}

        # bv broadcast across partitions: bvb[p, dv] = bv[dv]
        t = mix.tile([P, HPC, SH], F32, tag="ps", name="bv_ps")
        nc.tensor.matmul(t[:, 0, :], lhsT=ones_row[:, :P], rhs=bv_row,
                         start=True, stop=True)
        bvb_sb = pw.tile([P, D], F32, tag="bvb")
        nc.vector.tensor_copy(bvb_sb, t[:, 0, :])

        # qT first (needed by every score matmul)
        for cop in range(DC // 2):
            t = mix.tile([P, HPC, SH], F32, tag="ps", name=f"q_ps{cop}")
            for k in range(2):
                co = cop * 2 + k
                for ci in range(DC):
                    nc.tensor.matmul(t[:, k, :], lhsT=wq_sb[:, ci, ds(co * P, P)],
                                     rhs=xs16_sb[:, ci, :], start=(ci == 0), stop=(ci == DC - 1))
            for k in range(2):
                co = cop * 2 + k
                nc.scalar.activation(out=qT_sb[:, co, :], in_=t[:, k, :],
                                     func=AF.Identity, bias=bq_sb[:, co:co + 1])

        mask_dma(0)
        mask_dma(1)

        vstart = {h: True for h in range(H)}
        pend_av = []   # pending attnV work: (am_tile, tig, [h0, h1], outmap)

        def flush_attnV(keep=1):
            while len(pend_av) > keep:
                am_t, tig, heads2, outmap = pend_av.pop(0)
                for j, h in enumerate(heads2):
                    nc.tensor.matmul(outmap[h], lhsT=v_sb[:, tig, h, :],
                                     rhs=am_t[:, j, :],
                                     start=vstart[h], stop=(tig == TC - 1))
                    vstart[h] = False

        def attn_micro(tig, pr, heads, outmap, proj_fill, last=False):
            """One score pair + exp + mask + (lagged) attnV."""
            if pr == 0:
                mask_dma(tig + 2)
            sc = mix.tile([P, HPC, SH], F32, tag="ps", name=f"sc{tig}_{pr}")
            for half in range(HPC):
                h = heads[pr * HPC + half]
                c = h // HPC
                pslice = ds(half * Dh, Dh)
                nc.tensor.matmul(sc[:, half, :], lhsT=kT_sb[pslice, c, ds(tig * P, P)],
                                 rhs=qT_sb[pslice, c, :], start=True, stop=True,
                                 tile_position=(half * Dh, 0))
            if proj_fill is not None:
                proj_fill()
            am_t = amp.tile([P, HPC, SH], BF16, tag="am", name=f"am{tig}_{pr}")
            nc.scalar.activation(out=am_t, in_=sc, func=AF.Exp)
            m_t = mask_tiles[tig]
            for half in range(HPC):
                nc.vector.tensor_mul(am_t[:, half, :], am_t[:, half, :], m_t)
            pend_av.append((am_t, tig, [heads[pr * HPC], heads[pr * HPC + 1]], outmap))
            flush_attnV(keep=0 if last else 1)

        for tb in range(TB):
            if tb > 0:
                xt_cur = []
                for ci in range(DC):
                    t = p1x.tile([P, 512], DT16, tag=f"xt{ci}")
                    nc.sync.dma_start(out=t, in_=xT[ds(ci * P, P), ts(tb, 512)])
                    xt_cur.append(t)
            xt = xt_cur

            # 8 projection quarter-fills for this tb (4 kT chunks + 4 v chunks)
            quarters = [("k", co) for co in range(DC)] + [("v", tj) for tj in range(4)]

            def make_fill(kind, idx, xt=xt, tb=tb):
                def fill():
                    pj = mix.tile([P, HPC, SH], F32, tag="ps", name=f"pj{tb}_{kind}{idx}")
                    if kind == "k":
                        for ci in range(DC):
                            nc.tensor.matmul(pj[:, 0, :], lhsT=wk_sb[:, ci, ds(idx * P, P)],
                                             rhs=xt[ci], start=(ci == 0), stop=(ci == DC - 1))
                        nc.vector.tensor_scalar_add(kT_sb[:, idx, ts(tb, 512)],
                                                    pj[:, 0, :], bk_sb[:, idx:idx + 1])
                    else:
                        for ci in range(DC):
                            nc.tensor.matmul(pj[:, 0, :], lhsT=xt[ci][:, ds(idx * P, P)],
                                             rhs=wv_sb[:, ci, :], start=(ci == 0), stop=(ci == DC - 1))
                        nc.vector.tensor_add(
                            out=v_sb[:, tb * 4 + idx, :, 0:Dh],
                            in0=pj[:, 0, :].rearrange("p (h d) -> p h d", h=H),
                            in1=bvb_sb.rearrange("p (h d) -> p h d", h=H))
                return fill

            if tb == 0:
                for kind, idx in quarters:
                    make_fill(kind, idx)()
            else:
                m = 0
                for i in range(4):
                    tig = (tb - 1) * 4 + i
                    for pr in range(2):
                        kind, idx = quarters[m]
                        attn_micro(tig, pr, headsA, out_ps, make_fill(kind, idx))
                        m += 1

        # tail: attention for the last t-block (no proj filler)
        for i in range(4):
            tig = (TB - 1) * 4 + i
            for pr in range(2):
                attn_micro(tig, pr, headsA, out_ps, None, last=(i == 3 and pr == 1))

        # -------- epilogue A: normalize heads 0-3 into attn_sb --------
        for h in headsA:
            c, half = h // HPC, h % HPC
            rec_r = p2s.tile([1, SH], DT, tag="rec_r")
            with nc.allow_low_precision(reason="fp32 storage"):
                nc.vector.reciprocal(rec_r, out_ps[h][Dh:Dh + 1, :])
            bc = mix.tile([P, HPC, SH], F32, tag="ps", name=f"bcA{h}")
            nc.tensor.matmul(bc[0:Dh, 0, :], lhsT=ones_row[:1, :Dh],
                             rhs=rec_r, start=True, stop=True)
            bc_sb = p2s.tile([Dh, SH], DT, tag="bcsb")
            nc.scalar.copy(bc_sb, bc[0:Dh, 0, :])
            nc.vector.tensor_mul(attn_sb[ds(half * Dh, Dh), c, :],
                                 out_ps[h][0:Dh, :], bc_sb)

        outA_ctx.__exit__(None, None, None)

        # ================= pass B: heads 4-7 (ACT-bound; prefetch under it) ===
        headsB = [4, 5, 6, 7]
        wo_v = io["wo"].rearrange("(c p) n -> p c n", p=P)
        w1_v = io["w1"].rearrange("(c p) n -> p c n", p=P)
        w2_v = io["w2"].rearrange("(c p) n -> p c n", p=P)
        mask_tiles.clear()

        outB_ctx = tc.tile_pool(name="outB", bufs=1, space="PSUM")
        outB = outB_ctx.__enter__()
        out_psB = {h: outB.tile([Dh + 1, SH], F32, tag=f"outB{h}", name=f"out_psB{h}")
                   for h in headsB}

        # residual pre-adds + phase-3 weight prefetches ride under pass B
        xsb = [pbr.tile([P, SH], DT, tag=f"xsb{c}", name=f"xsb{c}") for c in range(DC)]
        xr = [pbr.tile([P, SH], DT, tag=f"xr{c}", name=f"xr{c}") for c in range(DC)]
        wo_t = pbr.tile([P, DC, D], DT16, tag="wo")
        nc.sync.dma_start(out=wo_t, in_=wo_v)
        w1_t = pbr.tile([P, DC, F], DT16, tag="w1")
        nc.sync.dma_start(out=w1_t, in_=w1_v)

        mask_dma(0)
        mask_dma(1)
        for tig in range(TC):
            for pr in range(2):
                attn_micro(tig, pr, headsB, out_psB, None,
                           last=(tig == TC - 1 and pr == 1))
            if tig < DC:
                nc.vector.tensor_scalar_add(xsb[tig], xs_sb[:, tig, :],
                                            bo_sb[:, tig:tig + 1])

        # -------- epilogue B --------
        for h in headsB:
            c, half = h // HPC, h % HPC
            rec_r = p2s.tile([1, SH], DT, tag="rec_r")
            with nc.allow_low_precision(reason="fp32 storage"):
                nc.vector.reciprocal(rec_r, out_psB[h][Dh:Dh + 1, :])
            bc = mix.tile([P, HPC, SH], F32, tag="ps", name=f"bcB{h}")
            nc.tensor.matmul(bc[0:Dh, 0, :], lhsT=ones_row[:1, :Dh],
                             rhs=rec_r, start=True, stop=True)
            bc_sb = p2s.tile([Dh, SH], DT, tag="bcsb")
            nc.scalar.copy(bc_sb, bc[0:Dh, 0, :])
            nc.vector.tensor_mul(attn_sb[ds(half * Dh, Dh), c, :],
                                 out_psB[h][0:Dh, :], bc_sb)

        outB_ctx.__exit__(None, None, None)
        mix_ctx.__exit__(None, None, None)
        mid.close()   # frees kT/qT/v, proj weights, am/mask/xt pools

        # ================= phase 3: out proj + LN1 + FFN + LN2 ===============
        p3big_ctx = tc.tile_pool(name="p3big", bufs=1)
        p3big = p3big_ctx.__enter__()
        with tc.tile_pool(name="p3", bufs=2) as p3, \
             tc.tile_pool(name="p3ps", bufs=2, space="PSUM") as p3ps, \
             tc.tile_pool(name="p3st", bufs=1, space="PSUM") as p3st, \
             tc.tile_pool(name="p3bc", bufs=2, space="PSUM") as p3bc:

            def ln_finalize(mu_ps, m2_ps):
                mu_s = p3.tile([1, SH], DT, tag="mu_s")
                m2_s = p3.tile([1, SH], DT, tag="m2_s")
                nc.vector.tensor_scalar_mul(mu_s, mu_ps, -1.0 / D)  # negated mean
                nc.vector.tensor_scalar_mul(m2_s, m2_ps, 1.0 / D)
                var_s = p3.tile([1, SH], DT, tag="var_s")
                nc.vector.tensor_mul(var_s, mu_s, mu_s)
                nc.vector.tensor_sub(var_s, m2_s, var_s)
                sd_s = p3.tile([1, SH], F32, tag="sd_s")
                nc.scalar.activation(out=sd_s, in_=var_s, func=AF.Sqrt, bias=eps_sb)
                rstd_s = p3.tile([1, SH], DT, tag="rstd_s")
                with nc.allow_low_precision(reason="fp32 storage"):
                    nc.vector.reciprocal(rstd_s, sd_s)
                off_s = p3.tile([1, SH], DT, tag="off_s")
                nc.vector.tensor_mul(off_s, mu_s, rstd_s)
                return rstd_s, off_s

            def ln_apply(src_c, g_row, be_row, rstd_s, off_s, dst_c, c):
                sc_b = p3bc.tile([P, SH], F32, tag="sc_b")
                of_b = p3bc.tile([P, SH], F32, tag="of_b")
                nc.tensor.matmul(sc_b, lhsT=g_row[:, ds(c * P, P)], rhs=rstd_s,
                                 start=True, stop=True)
                nc.tensor.matmul(of_b, lhsT=g_row[:, ds(c * P, P)], rhs=off_s,
                                 start=True, stop=False)
                nc.tensor.matmul(of_b, lhsT=be_row[:, ds(c * P, P)], rhs=ones_s,
                                 start=False, stop=True)
                t = p3.tile([P, SH], DT, tag="lnt")
                nc.vector.tensor_mul(t, src_c, sc_b)
                nc.vector.tensor_add(dst_c, t, of_b)

            # ---- out projection + residual, LN1 stats interleaved ----
            mu1 = p3st.tile([1, SH], F32, tag="mu")
            m21 = p3st.tile([1, SH], F32, tag="m2")
            for co in range(DC):
                ps = p3ps.tile([P, SH], F32, tag="ps")
                for ci in range(DC):
                    nc.tensor.matmul(ps, lhsT=wo_t[:, ci, ds(co * P, P)],
                                     rhs=attn_sb[:, ci, :],
                                     start=(ci == 0), stop=(ci == DC - 1))
                nc.vector.tensor_add(xr[co], ps, xsb[co])
                nc.tensor.matmul(mu1, lhsT=ones_col, rhs=xr[co],
                                 start=(co == 0), stop=(co == DC - 1))
                sq = p3.tile([P, SH], DT, tag="sq")
                nc.scalar.activation(out=sq, in_=xr[co], func=AF.Square)
                nc.tensor.matmul(m21, lhsT=ones_col, rhs=sq,
                                 start=(co == 0), stop=(co == DC - 1))

            rstd1, off1 = ln_finalize(mu1, m21)
            x1 = [p3big.tile([P, SH], DT, tag=f"x1{c}", name=f"x1{c}") for c in range(DC)]
            x1h = [p3big.tile([P, SH], DT16, tag=f"x1h{c}", name=f"x1h{c}") for c in range(DC)]
            for c in range(DC):
                ln_apply(xr[c], g1_row, be1_row, rstd1, off1, x1[c], c)
                nc.scalar.copy(x1h[c], x1[c])

            # ---- FFN ----
            w2_t = p3big.tile([P, FC, D], DT16, tag="w2")
            nc.sync.dma_start(out=w2_t, in_=w2_v)
            hT = p3big.tile([P, FC, SH], DT16, tag="hT")
            for fc in range(FC):
                ps = p3ps.tile([P, SH], F32, tag="ps")
                for ci in range(DC):
                    nc.tensor.matmul(ps, lhsT=w1_t[:, ci, ds(fc * P, P)],
                                     rhs=x1h[ci], start=(ci == 0), stop=(ci == DC - 1))
                nc.scalar.activation(out=hT[:, fc, :], in_=ps, func=AF.Relu,
                                     bias=b1_sb[:, fc:fc + 1])

            x1b = [p3big.tile([P, SH], DT, tag=f"x1b{c}", name=f"x1b{c}") for c in range(DC)]
            for c in range(DC):
                nc.vector.tensor_scalar_add(x1b[c], x1[c], b2_sb[:, c:c + 1])

            mu2 = p3st.tile([1, SH], F32, tag="mu", name="mu2")
            m22 = p3st.tile([1, SH], F32, tag="m2", name="m22")
            xr2 = [p3big.tile([P, SH], DT, tag=f"xr2{c}", name=f"xr2{c}") for c in range(DC)]
            for co in range(DC):
                ps = p3ps.tile([P, SH], F32, tag="ps")
                for fc in range(FC):
                    nc.tensor.matmul(ps, lhsT=w2_t[:, fc, ds(co * P, P)],
                                     rhs=hT[:, fc, :], start=(fc == 0), stop=(fc == FC - 1))
                nc.vector.tensor_add(xr2[co], ps, x1b[co])
                nc.tensor.matmul(mu2, lhsT=ones_col, rhs=xr2[co],
                                 start=(co == 0), stop=(co == DC - 1))
                sq = p3.tile([P, SH], DT, tag="sq")
                nc.scalar.activation(out=sq, in_=xr2[co], func=AF.Square)
                nc.tensor.matmul(m22, lhsT=ones_col, rhs=sq,
                                 start=(co == 0), stop=(co == DC - 1))

            rstd2, off2 = ln_finalize(mu2, m22)
            x2 = [p3big.tile([P, SH], F32, tag=f"x2{c}", name=f"x2{c}") for c in range(DC)]
            for c in range(DC):
                ln_apply(xr2[c], g2_row, be2_row, rstd2, off2, x2[c], c)
                nc.sync.dma_start(out=outT[ds(c * P, P), :], in_=x2[c])

        p3big_ctx.__exit__(None, None, None)


# ---------------------------------------------------------------------------
# host-side entry point
# ---------------------------------------------------------------------------

_CACHE = {}


def _get_compiled(S, D, F, H):
    key = (S, D, F, H)
    if key not in _CACHE:
        nc = bacc.Bacc("TRN2", target_bir_lowering=False, debug=False,
                       num_devices=N_CORES)
        build_encoder_kernel(nc, S=S, D=D, F=F, H=H, n_cores=N_CORES)
        nc.compile()
        _CACHE[key] = nc
    return _CACHE[key]


def make_in_maps(x, mask, weights, S, D, n_cores=N_CORES):
    """Shard + lay out inputs per core. x: (S, D) f32; mask: (S, S) int."""
    SH = S // n_cores
    xT = np.ascontiguousarray(x.T)                       # (D, S)
    maskb = (mask != 0)
    in_maps = []
    for c in range(n_cores):
        sl = slice(c * SH, (c + 1) * SH)
        im = {
            "xT": xT.astype(np.float16),
            "xsT": np.ascontiguousarray(xT[:, sl]),
            "xs16": np.ascontiguousarray(xT[:, sl]).astype(np.float16),
            "maskT": np.ascontiguousarray(maskb[sl, :].T).astype(ml_dtypes.bfloat16),
            "ones": np.ones(512, np.float32),
        }
        im.update({k: (v.astype(np.float16) if k in ("wq", "wk", "wv", "wo", "w1", "w2")
                       else v) for k, v in weights.items()})
        in_maps.append(im)
    return in_maps


def kernel(**inputs):
    x = np.asarray(inputs["x"], np.float32)
    mask = np.asarray(inputs["mask"])
    B, S, D = x.shape
    F = inputs["w1"].shape[1]
    H = 8
    assert B == 1
    weights = {k: np.asarray(inputs[k], np.float32)
               for k in ("wq", "wk", "wv", "wo", "w1", "w2",
                         "bq", "bk", "bv", "bo", "b1", "b2",
                         "g1", "be1", "g2", "be2")}
    nc = _get_compiled(S, D, F, H)
    in_maps = make_in_maps(x[0], mask, weights, S, D)
    res = run_bass_kernel_spmd(nc, in_maps, list(range(N_CORES)))
    SH = S // N_CORES
    out = np.empty((S, D), np.float32)
    for c in range(N_CORES):
        out[c * SH:(c + 1) * SH, :] = res.results[c]["outT"].T
    return out[None]


# revision 22
# speedup vs baseline: 1.0075x; 1.0075x over previous
"""Trainium2 Bass kernel for a dense transformer encoder layer.

Problem: B=1, S=4096, D=512, F=2048, H=8 heads (Dh=64), fp32 reference,
attention WITHOUT 1/sqrt(Dh) scaling, int mask (0 -> -1e9 before softmax),
two LayerNorms, ReLU FFN.

Sharding (query/row-parallel, no collectives): every core redundantly
computes the full kT = (x@wk).T and v = x@wv, plus its own 512-query shard.

Pipelined schedule (v2): the kT/v projections are software-pipelined INTO
attention pass A (heads 0-3): per 128-key chunk the PE queue interleaves
[score pair] [4 projection matmuls] so the ACT exp latency of the previous
score pair hides behind projection work and the PE never idles.  PSUM: 4
banks hold the 4 attnV accumulators (V gets a ones column so softmax
denominators are free), the other 4 banks are a 2-deep rotating pool shared
by score pairs and projection chunks.  Pass B (heads 4-7) is ACT-bound by
construction (exp is 1 elem/cycle/lane); it uses a 4-bank score tile so exp
runs as one [128,2048] instruction, and the FFN weight DMAs + residual
pre-adds ride in its shadow.  Softmax reciprocals and LayerNorm rstd use
the fast custom-DVE reciprocal (no giant single-lane RECIPROCAL ops, no
extra ACT table loads).  LayerNorm statistics matmuls are interleaved with
the producing projection loops.

dtypes: fp16 (tf32-grade) for QKV/FFN weights+activations and scores; bf16
for exp outputs / V / mask (exp needs the bf16 exponent range); fp32r for
K=1 broadcast matmuls and LN/residual arithmetic; fp32 accumulation in PSUM.
"""

import numpy as np
import ml_dtypes

import concourse.bass as bass
import concourse.bacc as bacc
import concourse.tile as tile
from concourse import mybir
from concourse.bass import ts, ds
from concourse.bass_utils import run_bass_kernel_spmd

AF = mybir.ActivationFunctionType
F32 = mybir.dt.float32
DT = mybir.dt.float32r  # fp32 storage, single-pass PE mode
DT16 = mybir.dt.float16
BF16 = mybir.dt.bfloat16

N_CORES = 8
EPS = 1e-5


def build_encoder_kernel(nc, S=4096, D=512, F=2048, H=8, n_cores=8):
    P = 128
    SH = S // n_cores          # query shard per core
    DC = D // P                # feature chunks of 128
    FC = F // P                # ffn chunks of 128
    TB = S // 512              # 512-wide t blocks
    TC = S // P                # 128-tall t chunks
    Dh = D // H
    assert Dh == 64 and DC * P == D

    d = lambda name, shape, dt: nc.dram_tensor(name, shape, dt, kind="ExternalInput").ap()
    xT = d("xT", [D, S], DT16)
    xsT = d("xsT", [D, SH], DT)
    xs16 = d("xs16", [D, SH], DT16)
    maskT = d("maskT", [S, SH], BF16)
    wq, wk, wv, wo = (d(n, [D, D], DT16) for n in ("wq", "wk", "wv", "wo"))
    w1 = d("w1", [D, F], DT16)
    w2 = d("w2", [F, D], DT16)
    bq, bk, bo = (d(n, [D], F32) for n in ("bq", "bk", "bo"))
    bv = d("bv", [D], DT)
    b1 = d("b1", [F], F32)
    b2 = d("b2", [D], F32)
    g1, be1, g2, be2 = (d(n, [D], DT) for n in ("g1", "be1", "g2", "be2"))
    ones = d("ones", [512], DT)
    outT = nc.dram_tensor("outT", [D, SH], F32, kind="ExternalOutput").ap()

    with tile.TileContext(nc) as tc:
        _emit(nc, tc, locals())


def _emit(nc, tc, io):
    P = 128
    xT, xsT, maskT = io["xT"], io["xsT"], io["maskT"]
    outT = io["outT"]
    S, D, F, H = io["S"], io["D"], io["F"], io["H"]
    SH, DC, FC, TB, TC, Dh = io["SH"], io["DC"], io["FC"], io["TB"], io["TC"], io["Dh"]
    HPC = P // Dh              # heads per 128-feature chunk (2)

    from contextlib import ExitStack
    with ExitStack() as root:
        # ---- global pools (live for the whole kernel) ----
        gconst = root.enter_context(tc.tile_pool(name="gconst", bufs=1))
        gbig = root.enter_context(tc.tile_pool(name="gbig", bufs=1))

        eps_sb = gconst.tile([1, 1], F32)
        nc.vector.memset(eps_sb, EPS)
        ones16_col = gconst.tile([P, 1], DT16)
        nc.vector.memset(ones16_col, 1.0)

        xs16_sb = gbig.tile([P, DC, SH], DT16)    # fp16 twin for the q projection
        nc.sync.dma_start(out=xs16_sb, in_=io["xs16"].rearrange("(c p) s -> p c s", p=P))
        xs_sb = gbig.tile([P, DC, SH], DT)        # own x shard (residual)
        attn_sb = gbig.tile([P, DC, SH], DT16)    # normalized attention out^T

        def load_vec(name, chunks):               # (n,) -> [128, chunks]
            t = gconst.tile([P, chunks], F32, tag=f"vec_{name}", name=f"vec_{name}")
            nc.sync.dma_start(out=t, in_=io[name].rearrange("(c p) -> p c", p=P))
            return t


        def load_row(name):                       # (n,) -> [1, n] f32r row
            t = gconst.tile([1, D], DT, tag=f"row_{name}", name=f"row_{name}")
            nc.sync.dma_start(out=t, in_=io[name][None, :])
            return t


        # pass-B riders / early phase-3 state (lives to the end; opened below
        # the mid pools so pool open/close stays LIFO)
        pbr = root.enter_context(tc.tile_pool(name="pbr", bufs=1))

        # ---- mid-lived pools: attention state + projection weights ----
        mid = ExitStack()
        kqv = mid.enter_context(tc.tile_pool(name="kqv", bufs=1))
        pw = mid.enter_context(tc.tile_pool(name="pw", bufs=1))
        p1x = mid.enter_context(tc.tile_pool(name="p1x", bufs=2))
        p2m = mid.enter_context(tc.tile_pool(name="p2m", bufs=3))
        amp = mid.enter_context(tc.tile_pool(name="amp", bufs=6))
        p2s = mid.enter_context(tc.tile_pool(name="p2s", bufs=3))
        epi = mid.enter_context(tc.tile_pool(name="epi", bufs=1))

        kT_sb = kqv.tile([P, DC, S], DT16)        # (x@wk)^T, full sequence
        qT_sb = kqv.tile([P, DC, SH], DT16)       # (xs@wq)^T
        v_sb = kqv.tile([P, TC, H, Dh + 1], BF16)  # v chunks + ones column
        nc.vector.memset(v_sb[:, :, :, Dh:Dh + 1], 1.0)

        # PE warmup on the first-arriving input
        with tc.tile_pool(name="warmps", bufs=1, space="PSUM") as warmps:
            wps = warmps.tile([1, SH], F32)
            for _ in range(8):
                nc.tensor.matmul(wps, lhsT=ones16_col, rhs=xs16_sb[:, 0, :],
                                 start=True, stop=True)

        def load_w(name):                         # (D, n) -> [128, DC, n]
            w = io[name]
            t = pw.tile([P, DC, w.shape[1]], DT16, tag=f"w_{name}", name=f"w_{name}")
            nc.sync.dma_start(out=t, in_=w.rearrange("(c p) n -> p c n", p=P))
            return t

        # first t-block of xT transfers early
        xt_cur = []
        for ci in range(DC):
            t = p1x.tile([P, 512], DT16, tag=f"xt{ci}", name=f"xt0_{ci}")
            nc.sync.dma_start(out=t, in_=xT[ds(ci * P, P), ts(0, 512)])
            xt_cur.append(t)

        wq_sb, wk_sb, wv_sb = load_w("wq"), load_w("wk"), load_w("wv")

        # remaining (non-critical-path) input DMAs
        nc.sync.dma_start(out=xs_sb, in_=xsT.rearrange("(c p) s -> p c s", p=P))
        ones_row = gconst.tile([1, P], DT)
        nc.sync.dma_start(out=ones_row, in_=io["ones"][None, :P])
        ones_col = gconst.tile([P, 1], DT)
        nc.sync.dma_start(out=ones_col, in_=io["ones"][:P, None])
        bq_sb, bk_sb, bo_sb = (load_vec(n, DC) for n in ("bq", "bk", "bo"))
        bv_row = gconst.tile([1, D], DT)
        nc.sync.dma_start(out=bv_row, in_=io["bv"][None, :])
        b1_sb = load_vec("b1", FC)
        b2_sb = load_vec("b2", DC)
        g1_row, be1_row, g2_row, be2_row = (load_row(n) for n in ("g1", "be1", "g2", "be2"))
        ones_s = gconst.tile([1, SH], DT)
        nc.sync.dma_start(out=ones_s, in_=io["ones"][None, :SH])

        mask_tiles = {}

        def mask_dma(tig):
            if tig in mask_tiles or tig >= TC:
                return
            m = p2m.tile([P, SH], BF16, tag="mask")
            nc.sync.dma_start(out=m, in_=maskT[ds(tig * P, P), :])
            mask_tiles[tig] = m

        # ============ merged phase: qT/kT/v projections + attention pass A ====
        mix_ctx = tc.tile_pool(name="mixps", bufs=2, space="PSUM")
        mix = mix_ctx.__enter__()
        outA_ctx = tc.tile_pool(name="outA", bufs=1, space="PSUM")
        outA = outA_ctx.__enter__()
        headsA = [0, 1, 2, 3]
        out_ps = {h: outA.tile([Dh + 1, SH], F32, tag=f"out{h}", name=f"out_ps{h}")
                  for h in headsA}

        # bv broadcast across partitions: bvb[p, dv] = bv[dv]
        t = mix.tile([P, HPC, SH], F32, tag="ps", name="bv_ps")
        nc.tensor.matmul(t[:, 0, :], lhsT=ones_row[:, :P], rhs=bv_row,
                         start=True, stop=True)
        bvb_sb = pw.tile([P, D], F32, tag="bvb")
        nc.vector.tensor_copy(bvb_sb, t[:, 0, :])

        # qT first (needed by every score matmul)
        for cop in range(DC // 2):
            t = mix.tile([P, HPC, SH], F32, tag="ps", name=f"q_ps{cop}")
            for k in range(2):
                co = cop * 2 + k
                for ci in range(DC):
                    nc.tensor.matmul(t[:, k, :], lhsT=wq_sb[:, ci, ds(co * P, P)],
                                     rhs=xs16_sb[:, ci, :], start=(ci == 0), stop=(ci == DC - 1))
            for k in range(2):
                co = cop * 2 + k
                nc.scalar.activation(out=qT_sb[:, co, :], in_=t[:, k, :],
                                     func=AF.Identity, bias=bq_sb[:, co:co + 1])

        mask_dma(0)
        mask_dma(1)

        vstart = {h: True for h in range(H)}
        pend_av = []   # pending attnV work: (am_tile, tig, [h0, h1], outmap)

        def flush_attnV(keep=1):
            while len(pend_av) > keep:
                am_t, tig, heads2, outmap = pend_av.pop(0)
                for j, h in enumerate(heads2):
                    nc.tensor.matmul(outmap[h], lhsT=v_sb[:, tig, h, :],
                                     rhs=am_t[:, j, :],
                                     start=vstart[h], stop=(tig == TC - 1))
                    vstart[h] = False

        def attn_micro(tig, pr, heads, outmap, proj_fill, last=False, keep=None):
            """One score pair + exp + mask + (lagged) attnV."""
            if pr == 0:
                mask_dma(tig + 2)
            sc = mix.tile([P, HPC, SH], F32, tag="ps", name=f"sc{tig}_{pr}")
            for half in range(HPC):
                h = heads[pr * HPC + half]
                c = h // HPC
                pslice = ds(half * Dh, Dh)
                nc.tensor.matmul(sc[:, half, :], lhsT=kT_sb[pslice, c, ds(tig * P, P)],
                                 rhs=qT_sb[pslice, c, :], start=True, stop=True,
                                 tile_position=(half * Dh, 0))
            if proj_fill is not None:
                proj_fill()
            am_t = amp.tile([P, HPC, SH], BF16, tag="am", name=f"am{tig}_{pr}")
            nc.scalar.activation(out=am_t, in_=sc, func=AF.Exp)
            m_t = mask_tiles[tig]
            for half in range(HPC):
                nc.vector.tensor_mul(am_t[:, half, :], am_t[:, half, :], m_t)
            pend_av.append((am_t, tig, [heads[pr * HPC], heads[pr * HPC + 1]], outmap))
            flush_attnV(keep=(0 if last else 1) if keep is None else keep)

        for tb in range(TB):
            if tb > 0:
                xt_cur = []
                for ci in range(DC):
                    t = p1x.tile([P, 512], DT16, tag=f"xt{ci}")
                    nc.sync.dma_start(out=t, in_=xT[ds(ci * P, P), ts(tb, 512)])
                    xt_cur.append(t)
            xt = xt_cur

            # 8 projection quarter-fills for this tb (4 kT chunks + 4 v chunks)
            quarters = [("k", co) for co in range(DC)] + [("v", tj) for tj in range(4)]

            def make_fill(kind, idx, xt=xt, tb=tb):
                def fill():
                    pj = mix.tile([P, HPC, SH], F32, tag="ps", name=f"pj{tb}_{kind}{idx}")
                    if kind == "k":
                        for ci in range(DC):
                            nc.tensor.matmul(pj[:, 0, :], lhsT=wk_sb[:, ci, ds(idx * P, P)],
                                             rhs=xt[ci], start=(ci == 0), stop=(ci == DC - 1))
                        nc.vector.tensor_scalar_add(kT_sb[:, idx, ts(tb, 512)],
                                                    pj[:, 0, :], bk_sb[:, idx:idx + 1])
                    else:
                        for ci in range(DC):
                            nc.tensor.matmul(pj[:, 0, :], lhsT=xt[ci][:, ds(idx * P, P)],
                                             rhs=wv_sb[:, ci, :], start=(ci == 0), stop=(ci == DC - 1))
                        nc.vector.tensor_add(
                            out=v_sb[:, tb * 4 + idx, :, 0:Dh],
                            in0=pj[:, 0, :].rearrange("p (h d) -> p h d", h=H),
                            in1=bvb_sb.rearrange("p (h d) -> p h d", h=H))
                return fill

            if tb == 0:
                for kind, idx in quarters:
                    make_fill(kind, idx)()
            else:
                m = 0
                for i in range(4):
                    tig = (tb - 1) * 4 + i
                    for pr in range(2):
                        kind, idx = quarters[m]
                        attn_micro(tig, pr, headsA, out_ps, make_fill(kind, idx))
                        m += 1

        # tail: attention for the last t-block (no proj filler)
        for i in range(4):
            tig = (TB - 1) * 4 + i
            for pr in range(2):
                attn_micro(tig, pr, headsA, out_ps, None, last=(i == 3 and pr == 1))

        epi_sums = {}
        epi_raw = {}

        def epilogue_copy(h, outmap):
            """Drain one head's PSUM (sums row + raw V-agg) to SBUF so the
            bank can be released; the normalize floats over later work."""
            s = epi.tile([1, SH], F32, tag=f"sums{h % 4}", name=f"sums{h}")
            nc.scalar.copy(s, outmap[h][Dh:Dh + 1, :])
            r = epi.tile([Dh, SH], F32, tag=f"raw{h % 4}", name=f"raw{h}")
            nc.scalar.copy(r, outmap[h][0:Dh, :])
            epi_sums[h], epi_raw[h] = s, r

        def epilogue_norm(h, tagpfx):
            c, half = h // HPC, h % HPC
            rec_r = p2s.tile([1, SH], DT, tag="rec_r")
            with nc.allow_low_precision(reason="fp32 storage"):
                nc.vector.reciprocal(rec_r, epi_sums[h])
            bc = mix.tile([P, HPC, SH], F32, tag="ps", name=f"bc{tagpfx}{h}")
            nc.tensor.matmul(bc[0:Dh, 0, :], lhsT=ones_row[:1, :Dh],
                             rhs=rec_r, start=True, stop=True)
            bc_sb = p2s.tile([Dh, SH], DT, tag="bcsb")
            nc.scalar.copy(bc_sb, bc[0:Dh, 0, :])
            nc.vector.tensor_mul(attn_sb[ds(half * Dh, Dh), c, :],
                                 epi_raw[h], bc_sb)

        # epilogue A: drain PSUM to SBUF (cheap ACT copies), release the banks;
        # the reciprocal+normalize rides under pass B's first chunks
        for h in headsA:
            epilogue_copy(h, out_ps)

        outA_ctx.__exit__(None, None, None)

        # ================= pass B: heads 4-7 (ACT-bound; prefetch under it) ===
        headsB = [4, 5, 6, 7]
        wo_v = io["wo"].rearrange("(c p) n -> p c n", p=P)
        w1_v = io["w1"].rearrange("(c p) n -> p c n", p=P)
        w2_v = io["w2"].rearrange("(c p) n -> p c n", p=P)
        mask_tiles.clear()

        outB_ctx = tc.tile_pool(name="outB", bufs=1, space="PSUM")
        outB = outB_ctx.__enter__()
        out_psB = {h: outB.tile([Dh + 1, SH], F32, tag=f"outB{h}", name=f"out_psB{h}")
                   for h in headsB}

        # residual pre-adds + phase-3 weight prefetches ride under pass B
        xsb = [pbr.tile([P, SH], DT, tag=f"xsb{c}", name=f"xsb{c}") for c in range(DC)]
        xr = [pbr.tile([P, SH], DT, tag=f"xr{c}", name=f"xr{c}") for c in range(DC)]
        wo_t = pbr.tile([P, DC, D], DT16, tag="wo")
        nc.sync.dma_start(out=wo_t, in_=wo_v)
        w1_t = pbr.tile([P, DC, F], DT16, tag="w1")
        nc.sync.dma_start(out=w1_t, in_=w1_v)

        mask_dma(0)
        mask_dma(1)
        for tig in range(TC):
            for pr in range(2):
                attn_micro(tig, pr, headsB, out_psB, None,
                           last=(tig == TC - 1 and pr == 1))
            if tig < DC:
                epilogue_norm(headsA[tig], "A")
                nc.vector.tensor_scalar_add(xsb[tig], xs_sb[:, tig, :],
                                            bo_sb[:, tig:tig + 1])

        # -------- epilogue B interleaved with the out-projection --------
        op_tiles = {}

        def outproj_pair(cop, cis):
            if cop not in op_tiles:
                op_tiles[cop] = opps.tile([P, HPC, SH], F32, tag=f"op{cop}", name=f"op{cop}")
            t = op_tiles[cop]
            for k in range(2):
                co = cop * 2 + k
                for ci in cis:
                    nc.tensor.matmul(t[:, k, :], lhsT=wo_t[:, ci, ds(co * P, P)],
                                     rhs=attn_sb[:, ci, :],
                                     start=(ci == 0), stop=(ci == DC - 1))

        for h in headsB:
            epilogue_copy(h, out_psB)
        outB_ctx.__exit__(None, None, None)
        opps_ctx = tc.tile_pool(name="opps", bufs=1, space="PSUM")
        opps = opps_ctx.__enter__()

        epilogue_norm(4, "B")
        epilogue_norm(5, "B")
        outproj_pair(0, (0, 1, 2))
        outproj_pair(1, (0, 1, 2))
        epilogue_norm(6, "B")
        epilogue_norm(7, "B")
        outproj_pair(0, (3,))
        outproj_pair(1, (3,))
        for cop in range(DC // 2):
            for k in range(2):
                co = cop * 2 + k
                nc.vector.tensor_add(xr[co], op_tiles[cop][:, k, :], xsb[co])

        opps_ctx.__exit__(None, None, None)
        mix_ctx.__exit__(None, None, None)
        mid.close()   # frees kT/qT/v, proj weights, am/mask/xt pools

        # ================= phase 3: out proj + LN1 + FFN + LN2 ===============
        p3big_ctx = tc.tile_pool(name="p3big", bufs=1)
        p3big = p3big_ctx.__enter__()
        with tc.tile_pool(name="p3", bufs=2) as p3, \
             tc.tile_pool(name="p3ps", bufs=2, space="PSUM") as p3ps, \
             tc.tile_pool(name="p3st", bufs=1, space="PSUM") as p3st, \
             tc.tile_pool(name="p3bc", bufs=2, space="PSUM") as p3bc:

            def ln_finalize(mu_ps, m2_ps):
                mu_s = p3.tile([1, SH], DT, tag="mu_s")
                m2_s = p3.tile([1, SH], DT, tag="m2_s")
                nc.vector.tensor_scalar_mul(mu_s, mu_ps, -1.0 / D)  # negated mean
                nc.vector.tensor_scalar_mul(m2_s, m2_ps, 1.0 / D)
                var_s = p3.tile([1, SH], DT, tag="var_s")
                nc.vector.tensor_mul(var_s, mu_s, mu_s)
                nc.vector.tensor_sub(var_s, m2_s, var_s)
                sd_s = p3.tile([1, SH], F32, tag="sd_s")
                nc.scalar.activation(out=sd_s, in_=var_s, func=AF.Sqrt, bias=eps_sb)
                rstd_s = p3.tile([1, SH], DT, tag="rstd_s")
                with nc.allow_low_precision(reason="fp32 storage"):
                    nc.vector.reciprocal(rstd_s, sd_s)
                off_s = p3.tile([1, SH], DT, tag="off_s")
                nc.vector.tensor_mul(off_s, mu_s, rstd_s)
                return rstd_s, off_s

            def ln_apply(src_c, g_row, be_row, rstd_s, off_s, dst_c, c):
                sc_b = p3bc.tile([P, SH], F32, tag="sc_b")
                of_b = p3bc.tile([P, SH], F32, tag="of_b")
                nc.tensor.matmul(sc_b, lhsT=g_row[:, ds(c * P, P)], rhs=rstd_s,
                                 start=True, stop=True)
                nc.tensor.matmul(of_b, lhsT=g_row[:, ds(c * P, P)], rhs=off_s,
                                 start=True, stop=False)
                nc.tensor.matmul(of_b, lhsT=be_row[:, ds(c * P, P)], rhs=ones_s,
                                 start=False, stop=True)
                t = p3.tile([P, SH], DT, tag="lnt")
                nc.vector.tensor_mul(t, src_c, sc_b)
                nc.vector.tensor_add(dst_c, t, of_b)

            # ---- LN1 stats (xr produced above, under epilogue B) ----
            mu1 = p3st.tile([1, SH], F32, tag="mu")
            m21 = p3st.tile([1, SH], F32, tag="m2")
            for co in range(DC):
                nc.tensor.matmul(mu1, lhsT=ones_col, rhs=xr[co],
                                 start=(co == 0), stop=(co == DC - 1))
                sq = p3.tile([P, SH], DT, tag="sq")
                nc.scalar.activation(out=sq, in_=xr[co], func=AF.Square)
                nc.tensor.matmul(m21, lhsT=ones_col, rhs=sq,
                                 start=(co == 0), stop=(co == DC - 1))

            rstd1, off1 = ln_finalize(mu1, m21)
            x1 = [p3big.tile([P, SH], DT, tag=f"x1{c}", name=f"x1{c}") for c in range(DC)]
            x1h = [p3big.tile([P, SH], DT16, tag=f"x1h{c}", name=f"x1h{c}") for c in range(DC)]
            for c in range(DC):
                ln_apply(xr[c], g1_row, be1_row, rstd1, off1, x1[c], c)
                nc.scalar.copy(x1h[c], x1[c])

            # ---- FFN ----
            w2_t = p3big.tile([P, FC, D], DT16, tag="w2")
            nc.sync.dma_start(out=w2_t, in_=w2_v)
            hT = p3big.tile([P, FC, SH], DT16, tag="hT")
            for fc in range(FC):
                ps = p3ps.tile([P, SH], F32, tag="ps")
                for ci in range(DC):
                    nc.tensor.matmul(ps, lhsT=w1_t[:, ci, ds(fc * P, P)],
                                     rhs=x1h[ci], start=(ci == 0), stop=(ci == DC - 1))
                nc.scalar.activation(out=hT[:, fc, :], in_=ps, func=AF.Relu,
                                     bias=b1_sb[:, fc:fc + 1])

            x1b = [p3big.tile([P, SH], DT, tag=f"x1b{c}", name=f"x1b{c}") for c in range(DC)]
            for c in range(DC):
                nc.vector.tensor_scalar_add(x1b[c], x1[c], b2_sb[:, c:c + 1])

            mu2 = p3st.tile([1, SH], F32, tag="mu", name="mu2")
            m22 = p3st.tile([1, SH], F32, tag="m2", name="m22")
            xr2 = [p3big.tile([P, SH], DT, tag=f"xr2{c}", name=f"xr2{c}") for c in range(DC)]
            for co in range(DC):
                ps = p3ps.tile([P, SH], F32, tag="ps")
                for fc in range(FC):
                    nc.tensor.matmul(ps, lhsT=w2_t[:, fc, ds(co * P, P)],
                                     rhs=hT[:, fc, :], start=(fc == 0), stop=(fc == FC - 1))
                nc.vector.tensor_add(xr2[co], ps, x1b[co])
                nc.tensor.matmul(mu2, lhsT=ones_col, rhs=xr2[co],
                                 start=(co == 0), stop=(co == DC - 1))
                sq = p3.tile([P, SH], DT, tag="sq")
                nc.scalar.activation(out=sq, in_=xr2[co], func=AF.Square)
                nc.tensor.matmul(m22, lhsT=ones_col, rhs=sq,
                                 start=(co == 0), stop=(co == DC - 1))

            rstd2, off2 = ln_finalize(mu2, m22)
            x2 = [p3big.tile([P, SH], F32, tag=f"x2{c}", name=f"x2{c}") for c in range(DC)]
            for c in range(DC):
                ln_apply(xr2[c], g2_row, be2_row, rstd2, off2, x2[c], c)
                nc.sync.dma_start(out=outT[ds(c * P, P), :], in_=x2[c])

        p3big_ctx.__exit__(None, None, None)


# ---------------------------------------------------------------------------
# host-side entry point
# ---------------------------------------------------------------------------

_CACHE = {}


def _get_compiled(S, D, F, H):
    key = (S, D, F, H)
    if key not in _CACHE:
        nc = bacc.Bacc("TRN2", target_bir_lowering=False, debug=False,
                       num_devices=N_CORES)
        build_encoder_kernel(nc, S=S, D=D, F=F, H=H, n_cores=N_CORES)
        nc.compile()
        _CACHE[key] = nc
    return _CACHE[key]


def make_in_maps(x, mask, weights, S, D, n_cores=N_CORES):
    """Shard + lay out inputs per core. x: (S, D) f32; mask: (S, S) int."""
    SH = S // n_cores
    xT = np.ascontiguousarray(x.T)                       # (D, S)
    maskb = (mask != 0)
    in_maps = []
    for c in range(n_cores):
        sl = slice(c * SH, (c + 1) * SH)
        im = {
            "xT": xT.astype(np.float16),
            "xsT": np.ascontiguousarray(xT[:, sl]),
            "xs16": np.ascontiguousarray(xT[:, sl]).astype(np.float16),
            "maskT": np.ascontiguousarray(maskb[sl, :].T).astype(ml_dtypes.bfloat16),
            "ones": np.ones(512, np.float32),
        }
        im.update({k: (v.astype(np.float16) if k in ("wq", "wk", "wv", "wo", "w1", "w2")
                       else v) for k, v in weights.items()})
        in_maps.append(im)
    return in_maps


def kernel(**inputs):
    x = np.asarray(inputs["x"], np.float32)
    mask = np.asarray(inputs["mask"])
    B, S, D = x.shape
    F = inputs["w1"].shape[1]
    H = 8
    assert B == 1
    weights = {k: np.asarray(inputs[k], np.float32)
               for k in ("wq", "wk", "wv", "wo", "w1", "w2",
                         "bq", "bk", "bv", "bo", "b1", "b2",
                         "g1", "be1", "g2", "be2")}
    nc = _get_compiled(S, D, F, H)
    in_maps = make_in_maps(x[0], mask, weights, S, D)
    res = run_bass_kernel_spmd(nc, in_maps, list(range(N_CORES)))
    SH = S // N_CORES
    out = np.empty((S, D), np.float32)
    for c in range(N_CORES):
        out[c * SH:(c + 1) * SH, :] = res.results[c]["outT"].T
    return out[None]
